# revision 46
# baseline (speedup 1.0000x reference)
"""AttentionRNNCell Trainium2 kernel (v2).

Math (per batch row b):
  et[t]  = V_a . tanh( (h W_a + b_a) + x[t] U_a )        t in [0, TE)
  at     = exp(et);  s = sum(at)
  ctx    = (sum_t at[t] x[t]) / s
  zt     = sigmoid(h W_z + [inp, ctx] C_z + b_z)
  rt     = sigmoid(h W_r + [inp, ctx] C_r + b_r)
  tht    = tanh((rt*h) U_p + [inp, ctx] C_p + b_p)
  ht     = (1-zt)*h + zt*tht

Distribution: data-parallel over batch B=128 across 8 cores (16 rows each).

v2 vs v1: the on-chip SBUF->SBUF DMA transposes of x (512 per core, ~635us
serialized through the Sync engine) are gone -- the host ships x twice in
bf16, pre-tiled in both layouts the PE needs:
  - xnat[b, p, tc, e] = x[b, tc*128+p, e]   (t on partitions, ctx rhs)
  - xtr [b, p, ec, t] = x[b, t, ec*128+p]   (e on partitions, uxpb rhs)
Context is computed with x as the MOVING operand (lhsT = at column, M=1,
N=256) -- 16 matmuls/row instead of 32 stationary x-tile loads + N=1
matmuls. Everything not depending on x_seq is folded on host as in v1.
"""

from contextlib import ExitStack

import numpy as np
import ml_dtypes

import concourse.bass as bass
import concourse.mybir as mybir
import concourse.tile as tile

BF16 = ml_dtypes.bfloat16
NPF8 = ml_dtypes.float8_e4m3
F32 = mybir.dt.float32
BF = mybir.dt.bfloat16
F8 = mybir.dt.float8e4
DR = mybir.MatmulPerfMode.DoubleRow
AF = mybir.ActivationFunctionType

B, TE, U, IN_DIM = 128, 2048, 256, 256
N_CORES = 8
BS = B // N_CORES  # 16 batch rows per core
P = 128
EC = U // P  # e-chunks (2)
UC = U // P  # u-chunks (2)


def split_multi_waits(nc, max_waits=1):
    """This container's walrus rejects instructions carrying more than one
    sync wait. Hoist extra waits onto standalone same-engine NoOps inserted
    immediately before the offending instruction (semantically identical:
    the engine blocks on each wait in order before executing it)."""
    n_new = 0
    for f in nc.m.functions:
        for blk in f.blocks:
            new_insts = []
            for inst in blk.instructions:
                si = inst.sync_info
                waits = list(si.on_wait) if si and si.on_wait else []
                if len(waits) > max_waits:
                    for w in waits[:-max_waits]:
                        nop = mybir.InstNoOp(
                            name=f"{inst.name}-hw{n_new}", ins=[], outs=[]
                        )
                        nop.engine = inst.engine
                        nop.sync_info = mybir.SyncInfo(on_wait=[w], on_update=[])
                        new_insts.append(nop)
                        n_new += 1
                    si.on_wait = waits[-max_waits:]
                new_insts.append(inst)
            blk.instructions = new_insts
    return n_new


def build_nc(bs=BS, te=TE, split_waits=True, debug_outs=False):
    """Build the per-core Bass module. Parametrized so a small variant can be
    simulated quickly; the production shape is (bs=16, te=2048)."""
    tc_n = te // P      # 128-col t-chunks (16)
    th_n = 2            # t halves (uxpb PSUM tile = [128, te/2] fp32, 2 banks)
    t_half = te // th_n
    tq_n = t_half // P  # 128-col chunks per half (8)

    nc = bass.Bass()
    xnat_d = nc.declare_dram_parameter("xnat", [bs // 2, P, tc_n, 2, U], F8, isOutput=False)
    xtr_d = nc.declare_dram_parameter("xtr", [bs, P, EC, te], F8, isOutput=False)
    ua_d = nc.declare_dram_parameter("ua", [U, U], F8, isOutput=False)
    va_d = nc.declare_dram_parameter("va", [U, 1], F8, isOutput=False)
    wxpbT_d = nc.declare_dram_parameter("wxpbT", [U, bs], F32, isOutput=False)
    hT_d = nc.declare_dram_parameter("hT", [U, bs], F32, isOutput=False)
    g0T_d = nc.declare_dram_parameter("g0T", [3, U, bs], F32, isOutput=False)
    cz_d = nc.declare_dram_parameter("cz", [U, U], F32, isOutput=False)
    cr_d = nc.declare_dram_parameter("cr", [U, U], F32, isOutput=False)
    cp_d = nc.declare_dram_parameter("cp", [U, U], F32, isOutput=False)
    up_d = nc.declare_dram_parameter("up", [U, U], F32, isOutput=False)
    id_d = nc.declare_dram_parameter("ident", [P, P], F32, isOutput=False)
    ht_d = nc.declare_dram_parameter("ht", [bs, U], F32, isOutput=True)
    if debug_outs:
        dbg_ctx_d = nc.declare_dram_parameter("dbg_ctx", [bs, U], F32, isOutput=True)
        dbg_es_d = nc.declare_dram_parameter("dbg_expsum", [P, bs], F32, isOutput=True)
        dbg_at_d = nc.declare_dram_parameter("dbg_at", [P, te // P], F32, isOutput=True)

    with tile.TileContext(nc) as tc, ExitStack() as ctx:
        singles = ctx.enter_context(tc.tile_pool(name="singles", bufs=1))
        xnat_p = ctx.enter_context(tc.tile_pool(name="xnat", bufs=4))
        xtr_p = ctx.enter_context(tc.tile_pool(name="xtr", bufs=3))
        tanh_p = ctx.enter_context(tc.tile_pool(name="tanh", bufs=8))
        at_p = ctx.enter_context(tc.tile_pool(name="at", bufs=4))
        small_p = ctx.enter_context(tc.tile_pool(name="small", bufs=4))
        uxpb_ps = ctx.enter_context(tc.tile_pool(name="uxpbps", bufs=3, space="PSUM"))
        et_ps = ctx.enter_context(tc.tile_pool(name="etps", bufs=1, space="PSUM"))
        ctx_ps = ctx.enter_context(tc.tile_pool(name="ctxps", bufs=1, space="PSUM"))

        # ---- setup: weights / small per-core tensors ----
        # Only ua/va/wxpb gate the first row's compute; everything else is
        # tail-only and loads on the (startup-idle) ACT queue after the
        # first rows' x DMAs are in flight.
        ua_sb = singles.tile([P, EC, U], F8)
        nc.sync.dma_start(out=ua_sb, in_=ua_d[:, :].rearrange("(c p) u -> p c u", p=P))
        va_sb = singles.tile([P, UC, 1], F8)
        nc.sync.dma_start(out=va_sb, in_=va_d[:, :].rearrange("(c p) o -> p c o", p=P))
        wxpb_sb = singles.tile([P, UC, bs], F32)
        nc.sync.dma_start(out=wxpb_sb, in_=wxpbT_d[:, :].rearrange("(c p) b -> p c b", p=P))
        hT_sb = singles.tile([P, UC, bs], F32)
        g0_sb = singles.tile([P, 3, UC, bs], F32)
        gate_w = {}
        for name in ("cz", "cr", "cp", "up"):
            gate_w[name] = singles.tile([P, EC, U], F32, name=f"{name}_sb")
        id_sb = singles.tile([P, P], F32)
        ones_sb = singles.tile([P, P], F32)
        nc.vector.memset(ones_sb, 1.0)

        def load_tail_weights():
            nc.sync.dma_start(out=hT_sb, in_=hT_d[:, :].rearrange("(c p) b -> p c b", p=P))
            nc.sync.dma_start(out=g0_sb, in_=g0T_d[:, :, :].rearrange("g (c p) b -> p g c b", p=P))
            for name, d in (("cz", cz_d), ("cr", cr_d), ("cp", cp_d), ("up", up_d)):
                nc.sync.dma_start(out=gate_w[name], in_=d[:, :].rearrange("(c p) u -> p c u", p=P))
            nc.sync.dma_start(out=id_sb, in_=id_d[:, :])
        expsum_all = singles.tile([P, bs], F32)
        ctx_rows = singles.tile([bs, U], F32)  # unnormalized ctx, one row per b
        # Block-diagonal at tiles for the paired-ctx DoubleRow: slot [j, m]
        # holds row (2q+j)'s at iff j == m, else stays the zero written once
        # here. Two tiles ping-pong across pairs.
        # [p, j, tc, m] layout: the k-tile (j) stride is tc_n*2 bytes -- the
        # dual-fp8 ldweights ISA requires k-tile stride >= 16 bytes.
        at2_tiles = []
        for i in range(2):
            at2 = singles.tile([P, 2, tc_n, 2], F8, name=f"at2_{i}")
            nc.vector.memset(at2, 0.0)
            at2_tiles.append(at2)

        # ---- streaming loop over batch rows, software-pipelined one deep.
        # Per iteration the issue order is: [et(b-1), exp(b-1)] ->
        # [uxpb(b), tanh(b)] -> [ctx(b-1)], so every PE instruction's
        # producer (ACT tanh/exp of the PREVIOUS row) has a full row of PE
        # work to hide behind -- the in-order PE queue never stalls.
        nchunk = te // 512  # xt ships as 4 chunk-tiles so each uxpb matmul
        # depends on exactly one DMA (deps are tile-granular)

        def stage_dma(b):
            xt = [
                xtr_p.tile([P, EC, 512], F8, tag=f"xtc{c}", name=f"xt{b}_{c}")
                for c in range(nchunk)
            ]
            for c in range(nchunk):
                src = xtr_d[b, :, :, c * 512 : (c + 1) * 512]
                if b == 0 and c == 1:
                    eng = nc.scalar  # startup-only third queue
                elif c % 2 == 0:
                    eng = nc.sync
                else:
                    eng = nc.gpsimd
                eng.dma_start(out=xt[c], in_=src)
            x_nat = None
            if b % 2 == 0:  # x for ctx lands pair-interleaved, one tile per pair
                x_nat = xnat_p.tile([P, tc_n, 2, U], F8, tag="xnat", name=f"xnat{b}")
                eng = nc.gpsimd if (b // 2) % 2 == 0 else nc.sync
                eng.dma_start(out=x_nat, in_=xnat_d[b // 2])
            return x_nat, xt

        def stage_uxpb_tanh(b, xt):
            # uxpb: out[u, t] = sum_e ua[e, u] * xt[e, t] -- fp8 DoubleRow
            # contracts both e-chunks in one matmul. tanh (per-partition
            # bias) -> SBUF fp8 tiles shaped [u, uc, t] for the et DoubleRow.
            tanh_ts = []
            for th in range(th_n):
                tanh_t = tanh_p.tile([P, UC, t_half], F8, tag="tanh", name=f"th{th}")
                n_mm = min(512, t_half)
                uxs = [
                    uxpb_ps.tile([P, t_half], F32, tag="ux", name=f"ux{uc}{th}")
                    for uc in range(UC)
                ]
                for n0 in range(0, t_half, n_mm):
                    chunk = (th * t_half + n0) // 512
                    for uc in range(UC):
                        nc.tensor.matmul(
                            out=uxs[uc][:, n0 : n0 + n_mm],
                            lhsT=ua_sb[:, :, uc * P : (uc + 1) * P],
                            rhs=xt[chunk],
                            perf_mode=DR,
                        )
                for uc in range(UC):
                    nc.scalar.activation(
                        out=tanh_t[:, uc, :], in_=uxs[uc], func=AF.Tanh,
                        bias=wxpb_sb[:, uc, b : b + 1],
                    )
                tanh_ts.append(tanh_t)
            return tanh_ts

        def stage_et_exp(b, tanh_ts):
            et = et_ps.tile([P, tc_n], F32, tag="etps", name=f"et{b}")
            for th in range(th_n):
                for tq in range(tq_n):
                    nc.tensor.matmul(
                        out=et[:, th * tq_n + tq : th * tq_n + tq + 1],
                        lhsT=tanh_ts[th][:, :, tq * P : (tq + 1) * P],
                        rhs=va_sb,
                        perf_mode=DR,
                    )
            # exp lands on the diagonal slot of the pair's block-diag tile
            j = b % 2
            at2 = at2_tiles[(b // 2) % 2]
            nc.scalar.activation(
                out=at2[:, j, :, j], in_=et, func=AF.Exp,
                accum_out=expsum_all[:, b : b + 1],
            )
            if debug_outs and b == 0:
                at_f32 = small_p.tile([P, tc_n], F32, name="at_f32")
                nc.vector.tensor_copy(at_f32, at2[:, 0, :, 0])
                nc.sync.dma_start(out=dbg_at_d[:, :], in_=at_f32)
            return at2

        def stage_ctx_pair(q, at2, x_nat):
            # Paired ctx: block-diagonal at2 on the two k-tiles against the
            # pair-interleaved x tile -> out[m, e] = row (2q+m)'s ctx partial.
            cps = ctx_ps.tile([2, U], F32, tag="ctxps", name=f"cps{q}")
            for tcc in range(tc_n):
                nc.tensor.matmul(
                    out=cps,
                    lhsT=at2[:, :, tcc, :],
                    rhs=x_nat[:, tcc, :, :],
                    start=(tcc == 0),
                    stop=(tcc == tc_n - 1),
                    perf_mode=DR,
                )
            stg = small_p.tile([2, U], F32, tag="ctxstg", name=f"stg{q}")
            nc.vector.tensor_copy(stg, cps)
            nc.sync.dma_start(out=ctx_rows[2 * q : 2 * q + 2, :], in_=stg)

        prev = None  # (b, tanh_ts)
        pair_xnat = {}
        for b in range(bs):
            x_nat, xt = stage_dma(b)
            if x_nat is not None:
                pair_xnat[b // 2] = x_nat
            if b == 1:
                load_tail_weights()
            if prev is not None:
                at2_prev = stage_et_exp(prev[0], prev[1])
                pb = prev[0]
            tanh_ts = stage_uxpb_tanh(b, xt)
            if prev is not None and pb % 2 == 1:
                stage_ctx_pair(pb // 2, at2_prev, pair_xnat.pop(pb // 2))
            prev = (b, tanh_ts)
        at2_last = stage_et_exp(prev[0], prev[1])
        stage_ctx_pair(prev[0] // 2, at2_last, pair_xnat.pop(prev[0] // 2))

        # ---- tail: normalize context, gates, output ----
        if debug_outs:
            nc.sync.dma_start(out=dbg_ctx_d[:, :], in_=ctx_rows)
            nc.sync.dma_start(out=dbg_es_d[:, :], in_=expsum_all)
        s_ps = et_ps.tile([P, bs], F32, tag="etps", name="s_ps")
        nc.tensor.matmul(out=s_ps, lhsT=ones_sb, rhs=expsum_all)
        recips = small_p.tile([P, bs], F32)
        nc.vector.reciprocal(recips, s_ps)
        # ctx_rows [bs, U] -> ctxT [e%128, ec, b] via PE transpose; normalize
        # by 1/s on the way out of PSUM.
        ctxn = singles.tile([P, EC, bs], F32)
        for e in range(EC):
            tp = et_ps.tile([P, bs], F32, tag="etps", name=f"ctxT{e}")
            nc.tensor.transpose(tp, ctx_rows[:, e * P : (e + 1) * P], id_sb[0:bs, 0:bs])
            nc.vector.tensor_mul(ctxn[:, e, :], tp, recips)

        def gate_psum(w_names_rhs, name):
            """psum[uc] = sum over (w, rhs) pairs of w^T @ rhs, per u-chunk."""
            outs = []
            for uc in range(UC):
                g = et_ps.tile([P, bs], F32, tag="etps", name=f"{name}{uc}")
                n_mm = sum(EC for _ in w_names_rhs)
                i = 0
                for w_sb, rhs_fn in w_names_rhs:
                    for e in range(EC):
                        nc.tensor.matmul(
                            out=g,
                            lhsT=w_sb[:, e, uc * P : (uc + 1) * P],
                            rhs=rhs_fn(e),
                            start=(i == 0),
                            stop=(i == n_mm - 1),
                        )
                        i += 1
                outs.append(g)
            return outs

        # zt^T, rt^T = sigmoid(g0 + C_*ctx^T ctx^T)
        zt_sb = small_p.tile([P, UC, bs], F32)
        rt_sb = small_p.tile([P, UC, bs], F32)
        for gi, (wname, dst) in enumerate((("cz", zt_sb), ("cr", rt_sb))):
            gps = gate_psum([(gate_w[wname], lambda e: ctxn[:, e, :])], wname)
            for uc in range(UC):
                tmp = small_p.tile([P, bs], F32, tag="gtmp", name=f"t{wname}{uc}")
                nc.vector.tensor_add(tmp, gps[uc], g0_sb[:, gi, uc, :])
                nc.scalar.activation(out=dst[:, uc, :], in_=tmp, func=AF.Sigmoid)

        # rh^T = rt^T * h^T ; tht^T = tanh(g0p + U_p^T rh^T + C_pctx^T ctx^T)
        rh_sb = small_p.tile([P, UC, bs], F32)
        for uc in range(UC):
            nc.vector.tensor_mul(rh_sb[:, uc, :], rt_sb[:, uc, :], hT_sb[:, uc, :])
        gps = gate_psum(
            [(gate_w["up"], lambda e: rh_sb[:, e, :]), (gate_w["cp"], lambda e: ctxn[:, e, :])],
            "cp",
        )
        ht_nat = small_p.tile([bs, U], F32)
        for uc in range(UC):
            tmp = small_p.tile([P, bs], F32, tag="gtmp", name=f"tp{uc}")
            nc.vector.tensor_add(tmp, gps[uc], g0_sb[:, 2, uc, :])
            tht = small_p.tile([P, bs], F32, tag="gtmp", name=f"tht{uc}")
            nc.scalar.activation(out=tht, in_=tmp, func=AF.Tanh)
            # ht^T = h^T + zt^T*(tht^T - h^T)
            nc.vector.tensor_sub(tht, tht, hT_sb[:, uc, :])
            nc.vector.tensor_mul(tht, tht, zt_sb[:, uc, :])
            nc.vector.tensor_add(tht, tht, hT_sb[:, uc, :])
            tp = et_ps.tile([bs, P], F32, tag="etps", name=f"htp{uc}")
            nc.tensor.transpose(tp, tht, id_sb)
            nc.vector.tensor_copy(ht_nat[:, uc * P : (uc + 1) * P], tp)
        nc.sync.dma_start(out=ht_d[:, :], in_=ht_nat)

    if split_waits:
        split_multi_waits(nc)
    return nc


def _host_prep(inputs, h_tm, V_a, W_a, U_a, b_a, C_z, W_z, b_z, C_r, W_r, b_r,
               C_p, U_p, b_p):
    """Fold everything not depending on x_seq into small per-core tensors."""
    wxpb = h_tm @ W_a + b_a                                # [B, U]
    g_z0 = h_tm @ W_z + inputs @ C_z[:IN_DIM] + b_z        # [B, U]
    g_r0 = h_tm @ W_r + inputs @ C_r[:IN_DIM] + b_r
    g_p0 = inputs @ C_p[:IN_DIM] + b_p
    shared = {
        "ua": np.ascontiguousarray(U_a.astype(NPF8)),
        "va": np.ascontiguousarray(V_a.reshape(U, 1).astype(NPF8)),
        "cz": np.ascontiguousarray(C_z[IN_DIM:].astype(np.float32)),
        "cr": np.ascontiguousarray(C_r[IN_DIM:].astype(np.float32)),
        "cp": np.ascontiguousarray(C_p[IN_DIM:].astype(np.float32)),
        "up": np.ascontiguousarray(U_p.astype(np.float32)),
        "ident": np.eye(P, dtype=np.float32),
    }
    per_core = []
    for c in range(N_CORES):
        s = slice(c * BS, (c + 1) * BS)
        per_core.append(
            {
                "wxpbT": np.ascontiguousarray(wxpb[s].T.astype(np.float32)),
                "hT": np.ascontiguousarray(h_tm[s].T.astype(np.float32)),
                "g0T": np.ascontiguousarray(
                    np.stack([g_z0[s].T, g_r0[s].T, g_p0[s].T]).astype(np.float32)
                ),
                **shared,
            }
        )
    return per_core


def _prep_x(x_core):
    """Pre-tile one core's x [bs, TE, U] into both fp8 layouts."""
    xb = x_core.astype(NPF8)
    tc_n = TE // P
    # xnat[q, p, tc, j, e] = x[2q+j, tc*128+p, e]  (pair-interleaved)
    xnat = np.ascontiguousarray(
        xb.reshape(BS // 2, 2, tc_n, P, U).transpose(0, 3, 2, 1, 4)
    )
    # xtr[b, p, ec, t] = x[b, t, ec*128+p]
    xtr = np.ascontiguousarray(
        xb.reshape(BS, TE, EC, P).transpose(0, 3, 2, 1)
    )
    return xnat, xtr


def build_in_maps(all_inputs):
    """Full host prep: dict of the reference's 16 inputs -> per-core in_maps."""
    args = {k: np.asarray(v, dtype=np.float32) for k, v in all_inputs.items()
            if k != "x_seq"}
    x_seq = np.asarray(all_inputs["x_seq"], dtype=np.float32)
    per_core = _host_prep(**args)
    in_maps = []
    for c in range(N_CORES):
        m = dict(per_core[c])
        m["xnat"], m["xtr"] = _prep_x(x_seq[c * BS : (c + 1) * BS])
        in_maps.append(m)
    return in_maps


def kernel(inputs, h_tm, x_seq, V_a, W_a, U_a, b_a, C_z, W_z, b_z,
           C_r, W_r, b_r, C_p, U_p, b_p):
    from concourse.bass_utils import run_bass_kernel_spmd

    in_maps = build_in_maps(dict(
        inputs=inputs, h_tm=h_tm, x_seq=x_seq, V_a=V_a, W_a=W_a, U_a=U_a,
        b_a=b_a, C_z=C_z, W_z=W_z, b_z=b_z, C_r=C_r, W_r=W_r, b_r=b_r,
        C_p=C_p, U_p=U_p, b_p=b_p))
    nc = build_nc()
    res = run_bass_kernel_spmd(nc, in_maps, core_ids=list(range(N_CORES)))
    return np.concatenate([res.results[c]["ht"] for c in range(N_CORES)], axis=0)


# revision 47
# speedup vs baseline: 1.1492x; 1.1492x over previous
"""AttentionRNNCell Trainium2 kernel (v2).

Math (per batch row b):
  et[t]  = V_a . tanh( (h W_a + b_a) + x[t] U_a )        t in [0, TE)
  at     = exp(et);  s = sum(at)
  ctx    = (sum_t at[t] x[t]) / s
  zt     = sigmoid(h W_z + [inp, ctx] C_z + b_z)
  rt     = sigmoid(h W_r + [inp, ctx] C_r + b_r)
  tht    = tanh((rt*h) U_p + [inp, ctx] C_p + b_p)
  ht     = (1-zt)*h + zt*tht

Distribution: data-parallel over batch B=128 across 8 cores (16 rows each).

v2 vs v1: the on-chip SBUF->SBUF DMA transposes of x (512 per core, ~635us
serialized through the Sync engine) are gone -- the host ships x twice in
bf16, pre-tiled in both layouts the PE needs:
  - xnat[b, p, tc, e] = x[b, tc*128+p, e]   (t on partitions, ctx rhs)
  - xtr [b, p, ec, t] = x[b, t, ec*128+p]   (e on partitions, uxpb rhs)
Context is computed with x as the MOVING operand (lhsT = at column, M=1,
N=256) -- 16 matmuls/row instead of 32 stationary x-tile loads + N=1
matmuls. Everything not depending on x_seq is folded on host as in v1.
"""

from contextlib import ExitStack

import numpy as np
import ml_dtypes

import concourse.bass as bass
import concourse.mybir as mybir
import concourse.tile as tile

BF16 = ml_dtypes.bfloat16
NPF8 = ml_dtypes.float8_e4m3
F32 = mybir.dt.float32
BF = mybir.dt.bfloat16
F8 = mybir.dt.float8e4
DR = mybir.MatmulPerfMode.DoubleRow
AF = mybir.ActivationFunctionType

B, TE, U, IN_DIM = 128, 2048, 256, 256
N_CORES = 8
BS = B // N_CORES  # 16 batch rows per core
P = 128
EC = U // P  # e-chunks (2)
UC = U // P  # u-chunks (2)


def split_multi_waits(nc, max_waits=1):
    """This container's walrus rejects instructions carrying more than one
    sync wait. Hoist extra waits onto standalone same-engine NoOps inserted
    immediately before the offending instruction (semantically identical:
    the engine blocks on each wait in order before executing it)."""
    n_new = 0
    for f in nc.m.functions:
        for blk in f.blocks:
            new_insts = []
            for inst in blk.instructions:
                si = inst.sync_info
                waits = list(si.on_wait) if si and si.on_wait else []
                if len(waits) > max_waits:
                    for w in waits[:-max_waits]:
                        nop = mybir.InstNoOp(
                            name=f"{inst.name}-hw{n_new}", ins=[], outs=[]
                        )
                        nop.engine = inst.engine
                        nop.sync_info = mybir.SyncInfo(on_wait=[w], on_update=[])
                        new_insts.append(nop)
                        n_new += 1
                    si.on_wait = waits[-max_waits:]
                new_insts.append(inst)
            blk.instructions = new_insts
    return n_new


def build_nc(bs=BS, te=TE, split_waits=True, debug_outs=False):
    """Build the per-core Bass module. Parametrized so a small variant can be
    simulated quickly; the production shape is (bs=16, te=2048)."""
    tc_n = te // P      # 128-col t-chunks (16)
    th_n = 2            # t halves (uxpb PSUM tile = [128, te/2] fp32, 2 banks)
    t_half = te // th_n
    tq_n = t_half // P  # 128-col chunks per half (8)

    nc = bass.Bass()
    xnat_d = nc.declare_dram_parameter("xnat", [bs // 2, P, tc_n, 2, U], F8, isOutput=False)
    xtr_d = nc.declare_dram_parameter("xtr", [bs, P, EC, te], F8, isOutput=False)
    ua_d = nc.declare_dram_parameter("ua", [U, U], F8, isOutput=False)
    va_d = nc.declare_dram_parameter("va", [U, 1], F8, isOutput=False)
    wxpbT_d = nc.declare_dram_parameter("wxpbT", [U, bs], F32, isOutput=False)
    hT_d = nc.declare_dram_parameter("hT", [U, bs], F32, isOutput=False)
    g0T_d = nc.declare_dram_parameter("g0T", [3, U, bs], F32, isOutput=False)
    cz_d = nc.declare_dram_parameter("cz", [U, U], F32, isOutput=False)
    cr_d = nc.declare_dram_parameter("cr", [U, U], F32, isOutput=False)
    cp_d = nc.declare_dram_parameter("cp", [U, U], F32, isOutput=False)
    up_d = nc.declare_dram_parameter("up", [U, U], F32, isOutput=False)
    id_d = nc.declare_dram_parameter("ident", [P, P], F32, isOutput=False)
    ht_d = nc.declare_dram_parameter("ht", [bs, U], F32, isOutput=True)
    if debug_outs:
        dbg_ctx_d = nc.declare_dram_parameter("dbg_ctx", [bs, U], F32, isOutput=True)
        dbg_es_d = nc.declare_dram_parameter("dbg_expsum", [P, bs], F32, isOutput=True)
        dbg_at_d = nc.declare_dram_parameter("dbg_at", [P, te // P], F32, isOutput=True)

    with tile.TileContext(nc) as tc, ExitStack() as ctx:
        singles = ctx.enter_context(tc.tile_pool(name="singles", bufs=1))
        xnat_p = ctx.enter_context(tc.tile_pool(name="xnat", bufs=4))
        xtr_p = ctx.enter_context(tc.tile_pool(name="xtr", bufs=3))
        tanh_p = ctx.enter_context(tc.tile_pool(name="tanh", bufs=8))
        at_p = ctx.enter_context(tc.tile_pool(name="at", bufs=4))
        small_p = ctx.enter_context(tc.tile_pool(name="small", bufs=4))
        uxpb_ps = ctx.enter_context(tc.tile_pool(name="uxpbps", bufs=3, space="PSUM"))
        et_ps = ctx.enter_context(tc.tile_pool(name="etps", bufs=1, space="PSUM"))
        ctx_ps = ctx.enter_context(tc.tile_pool(name="ctxps", bufs=1, space="PSUM"))

        # ---- setup: weights / small per-core tensors ----
        # Only ua/va/wxpb gate the first row's compute; everything else is
        # tail-only and loads on the (startup-idle) ACT queue after the
        # first rows' x DMAs are in flight.
        ua_sb = singles.tile([P, EC, U], F8)
        nc.sync.dma_start(out=ua_sb, in_=ua_d[:, :].rearrange("(c p) u -> p c u", p=P))
        va_sb = singles.tile([P, UC, 1], F8)
        nc.sync.dma_start(out=va_sb, in_=va_d[:, :].rearrange("(c p) o -> p c o", p=P))
        wxpb_sb = singles.tile([P, UC, bs], F32)
        nc.sync.dma_start(out=wxpb_sb, in_=wxpbT_d[:, :].rearrange("(c p) b -> p c b", p=P))
        hT_sb = singles.tile([P, UC, bs], F32)
        g0_sb = singles.tile([P, 3, UC, bs], F32)
        gate_w = {}
        for name in ("cz", "cr", "cp", "up"):
            gate_w[name] = singles.tile([P, EC, U], F32, name=f"{name}_sb")
        id_sb = singles.tile([P, P], F32)
        ones_sb = singles.tile([P, P], F32)
        nc.vector.memset(ones_sb, 1.0)

        def load_tail_weights():
            nc.sync.dma_start(out=hT_sb, in_=hT_d[:, :].rearrange("(c p) b -> p c b", p=P))
            nc.sync.dma_start(out=g0_sb, in_=g0T_d[:, :, :].rearrange("g (c p) b -> p g c b", p=P))
            for name, d in (("cz", cz_d), ("cr", cr_d), ("cp", cp_d), ("up", up_d)):
                nc.sync.dma_start(out=gate_w[name], in_=d[:, :].rearrange("(c p) u -> p c u", p=P))
            nc.sync.dma_start(out=id_sb, in_=id_d[:, :])
        expsum_all = singles.tile([P, bs], F32)
        ctx_rows = singles.tile([bs, U], F32)  # unnormalized ctx, one row per b
        # Block-diagonal at tiles for the paired-ctx DoubleRow: slot [j, m]
        # holds row (2q+j)'s at iff j == m, else stays the zero written once
        # here. Two tiles ping-pong across pairs.
        # [p, j, tc, m] layout: the k-tile (j) stride is tc_n*2 bytes -- the
        # dual-fp8 ldweights ISA requires k-tile stride >= 16 bytes.
        at2_tiles = []
        for i in range(2):
            at2 = singles.tile([P, 2, tc_n, 2], F8, name=f"at2_{i}")
            nc.vector.memset(at2, 0.0)
            at2_tiles.append(at2)

        # ---- streaming loop over batch rows, software-pipelined one deep.
        # Per iteration the issue order is: [et(b-1), exp(b-1)] ->
        # [uxpb(b), tanh(b)] -> [ctx(b-1)], so every PE instruction's
        # producer (ACT tanh/exp of the PREVIOUS row) has a full row of PE
        # work to hide behind -- the in-order PE queue never stalls.
        def stage_dma(b):
            xt = xtr_p.tile([P, EC, te], F8, tag="xt", name=f"xt{b}")
            if b == 0:
                # 3-way split across all DMA-capable queues: minimize the
                # startup latency before the first uxpb matmul can run.
                t3 = te // 4
                nc.sync.dma_start(out=xt[:, :, 0:t3], in_=xtr_d[b, :, :, 0:t3])
                nc.scalar.dma_start(out=xt[:, :, t3 : 2 * t3], in_=xtr_d[b, :, :, t3 : 2 * t3])
                nc.gpsimd.dma_start(out=xt[:, :, 2 * t3 : te], in_=xtr_d[b, :, :, 2 * t3 : te])
            else:
                nc.sync.dma_start(out=xt[:, :, 0:t_half], in_=xtr_d[b, :, :, 0:t_half])
                nc.gpsimd.dma_start(out=xt[:, :, t_half:te], in_=xtr_d[b, :, :, t_half:te])
            x_nat = None
            if b % 2 == 0:  # x for ctx lands pair-interleaved, one tile per pair
                x_nat = xnat_p.tile([P, tc_n, 2, U], F8, tag="xnat", name=f"xnat{b}")
                nc.gpsimd.dma_start(out=x_nat, in_=xnat_d[b // 2])
            return x_nat, xt

        def stage_uxpb_tanh(b, xt):
            # uxpb: out[u, t] = sum_e ua[e, u] * xt[e, t] -- fp8 DoubleRow
            # contracts both e-chunks in one matmul. tanh (per-partition
            # bias) -> SBUF fp8 tiles shaped [u, uc, t] for the et DoubleRow.
            tanh_ts = []
            for th in range(th_n):
                tanh_t = tanh_p.tile([P, UC, t_half], F8, tag="tanh", name=f"th{th}")
                n_mm = min(512, t_half)
                uxs = [
                    uxpb_ps.tile([P, t_half], F32, tag="ux", name=f"ux{uc}{th}")
                    for uc in range(UC)
                ]
                for n0 in range(0, t_half, n_mm):
                    for uc in range(UC):
                        nc.tensor.matmul(
                            out=uxs[uc][:, n0 : n0 + n_mm],
                            lhsT=ua_sb[:, :, uc * P : (uc + 1) * P],
                            rhs=xt[:, :, th * t_half + n0 : th * t_half + n0 + n_mm],
                            perf_mode=DR,
                        )
                for uc in range(UC):
                    nc.scalar.activation(
                        out=tanh_t[:, uc, :], in_=uxs[uc], func=AF.Tanh,
                        bias=wxpb_sb[:, uc, b : b + 1],
                    )
                tanh_ts.append(tanh_t)
            return tanh_ts

        def stage_et_exp(b, tanh_ts):
            et = et_ps.tile([P, tc_n], F32, tag="etps", name=f"et{b}")
            for th in range(th_n):
                for tq in range(tq_n):
                    nc.tensor.matmul(
                        out=et[:, th * tq_n + tq : th * tq_n + tq + 1],
                        lhsT=tanh_ts[th][:, :, tq * P : (tq + 1) * P],
                        rhs=va_sb,
                        perf_mode=DR,
                    )
            # exp lands on the diagonal slot of the pair's block-diag tile
            j = b % 2
            at2 = at2_tiles[(b // 2) % 2]
            nc.scalar.activation(
                out=at2[:, j, :, j], in_=et, func=AF.Exp,
                accum_out=expsum_all[:, b : b + 1],
            )
            if debug_outs and b == 0:
                at_f32 = small_p.tile([P, tc_n], F32, name="at_f32")
                nc.vector.tensor_copy(at_f32, at2[:, 0, :, 0])
                nc.sync.dma_start(out=dbg_at_d[:, :], in_=at_f32)
            return at2

        def stage_ctx_pair(q, at2, x_nat):
            # Paired ctx: block-diagonal at2 on the two k-tiles against the
            # pair-interleaved x tile -> out[m, e] = row (2q+m)'s ctx partial.
            cps = ctx_ps.tile([2, U], F32, tag="ctxps", name=f"cps{q}")
            for tcc in range(tc_n):
                nc.tensor.matmul(
                    out=cps,
                    lhsT=at2[:, :, tcc, :],
                    rhs=x_nat[:, tcc, :, :],
                    start=(tcc == 0),
                    stop=(tcc == tc_n - 1),
                    perf_mode=DR,
                )
            stg = small_p.tile([2, U], F32, tag="ctxstg", name=f"stg{q}")
            nc.vector.tensor_copy(stg, cps)
            nc.sync.dma_start(out=ctx_rows[2 * q : 2 * q + 2, :], in_=stg)

        prev = None  # (b, tanh_ts)
        pair_xnat = {}
        for b in range(bs):
            x_nat, xt = stage_dma(b)
            if x_nat is not None:
                pair_xnat[b // 2] = x_nat
            if b == 1:
                load_tail_weights()
            if prev is not None:
                at2_prev = stage_et_exp(prev[0], prev[1])
                pb = prev[0]
            tanh_ts = stage_uxpb_tanh(b, xt)
            if prev is not None and pb % 2 == 1:
                stage_ctx_pair(pb // 2, at2_prev, pair_xnat.pop(pb // 2))
            prev = (b, tanh_ts)
        at2_last = stage_et_exp(prev[0], prev[1])
        stage_ctx_pair(prev[0] // 2, at2_last, pair_xnat.pop(prev[0] // 2))

        # ---- tail: normalize context, gates, output ----
        if debug_outs:
            nc.sync.dma_start(out=dbg_ctx_d[:, :], in_=ctx_rows)
            nc.sync.dma_start(out=dbg_es_d[:, :], in_=expsum_all)
        s_ps = et_ps.tile([P, bs], F32, tag="etps", name="s_ps")
        nc.tensor.matmul(out=s_ps, lhsT=ones_sb, rhs=expsum_all)
        recips = small_p.tile([P, bs], F32)
        nc.vector.reciprocal(recips, s_ps)
        # ctx_rows [bs, U] -> ctxT [e%128, ec, b] via PE transpose; normalize
        # by 1/s on the way out of PSUM.
        ctxn = singles.tile([P, EC, bs], F32)
        for e in range(EC):
            tp = et_ps.tile([P, bs], F32, tag="etps", name=f"ctxT{e}")
            nc.tensor.transpose(tp, ctx_rows[:, e * P : (e + 1) * P], id_sb[0:bs, 0:bs])
            nc.vector.tensor_mul(ctxn[:, e, :], tp, recips)

        def gate_psum(w_names_rhs, name):
            """psum[uc] = sum over (w, rhs) pairs of w^T @ rhs, per u-chunk."""
            outs = []
            for uc in range(UC):
                g = et_ps.tile([P, bs], F32, tag="etps", name=f"{name}{uc}")
                n_mm = sum(EC for _ in w_names_rhs)
                i = 0
                for w_sb, rhs_fn in w_names_rhs:
                    for e in range(EC):
                        nc.tensor.matmul(
                            out=g,
                            lhsT=w_sb[:, e, uc * P : (uc + 1) * P],
                            rhs=rhs_fn(e),
                            start=(i == 0),
                            stop=(i == n_mm - 1),
                        )
                        i += 1
                outs.append(g)
            return outs

        # zt^T, rt^T = sigmoid(g0 + C_*ctx^T ctx^T)
        zt_sb = small_p.tile([P, UC, bs], F32)
        rt_sb = small_p.tile([P, UC, bs], F32)
        for gi, (wname, dst) in enumerate((("cz", zt_sb), ("cr", rt_sb))):
            gps = gate_psum([(gate_w[wname], lambda e: ctxn[:, e, :])], wname)
            for uc in range(UC):
                tmp = small_p.tile([P, bs], F32, tag="gtmp", name=f"t{wname}{uc}")
                nc.vector.tensor_add(tmp, gps[uc], g0_sb[:, gi, uc, :])
                nc.scalar.activation(out=dst[:, uc, :], in_=tmp, func=AF.Sigmoid)

        # rh^T = rt^T * h^T ; tht^T = tanh(g0p + U_p^T rh^T + C_pctx^T ctx^T)
        rh_sb = small_p.tile([P, UC, bs], F32)
        for uc in range(UC):
            nc.vector.tensor_mul(rh_sb[:, uc, :], rt_sb[:, uc, :], hT_sb[:, uc, :])
        gps = gate_psum(
            [(gate_w["up"], lambda e: rh_sb[:, e, :]), (gate_w["cp"], lambda e: ctxn[:, e, :])],
            "cp",
        )
        ht_nat = small_p.tile([bs, U], F32)
        for uc in range(UC):
            tmp = small_p.tile([P, bs], F32, tag="gtmp", name=f"tp{uc}")
            nc.vector.tensor_add(tmp, gps[uc], g0_sb[:, 2, uc, :])
            tht = small_p.tile([P, bs], F32, tag="gtmp", name=f"tht{uc}")
            nc.scalar.activation(out=tht, in_=tmp, func=AF.Tanh)
            # ht^T = h^T + zt^T*(tht^T - h^T)
            nc.vector.tensor_sub(tht, tht, hT_sb[:, uc, :])
            nc.vector.tensor_mul(tht, tht, zt_sb[:, uc, :])
            nc.vector.tensor_add(tht, tht, hT_sb[:, uc, :])
            tp = et_ps.tile([bs, P], F32, tag="etps", name=f"htp{uc}")
            nc.tensor.transpose(tp, tht, id_sb)
            nc.vector.tensor_copy(ht_nat[:, uc * P : (uc + 1) * P], tp)
        nc.sync.dma_start(out=ht_d[:, :], in_=ht_nat)

    if split_waits:
        split_multi_waits(nc)
    return nc


def _host_prep(inputs, h_tm, V_a, W_a, U_a, b_a, C_z, W_z, b_z, C_r, W_r, b_r,
               C_p, U_p, b_p):
    """Fold everything not depending on x_seq into small per-core tensors."""
    wxpb = h_tm @ W_a + b_a                                # [B, U]
    g_z0 = h_tm @ W_z + inputs @ C_z[:IN_DIM] + b_z        # [B, U]
    g_r0 = h_tm @ W_r + inputs @ C_r[:IN_DIM] + b_r
    g_p0 = inputs @ C_p[:IN_DIM] + b_p
    shared = {
        "ua": np.ascontiguousarray(U_a.astype(NPF8)),
        "va": np.ascontiguousarray(V_a.reshape(U, 1).astype(NPF8)),
        "cz": np.ascontiguousarray(C_z[IN_DIM:].astype(np.float32)),
        "cr": np.ascontiguousarray(C_r[IN_DIM:].astype(np.float32)),
        "cp": np.ascontiguousarray(C_p[IN_DIM:].astype(np.float32)),
        "up": np.ascontiguousarray(U_p.astype(np.float32)),
        "ident": np.eye(P, dtype=np.float32),
    }
    per_core = []
    for c in range(N_CORES):
        s = slice(c * BS, (c + 1) * BS)
        per_core.append(
            {
                "wxpbT": np.ascontiguousarray(wxpb[s].T.astype(np.float32)),
                "hT": np.ascontiguousarray(h_tm[s].T.astype(np.float32)),
                "g0T": np.ascontiguousarray(
                    np.stack([g_z0[s].T, g_r0[s].T, g_p0[s].T]).astype(np.float32)
                ),
                **shared,
            }
        )
    return per_core


def _prep_x(x_core):
    """Pre-tile one core's x [bs, TE, U] into both fp8 layouts."""
    xb = x_core.astype(NPF8)
    tc_n = TE // P
    # xnat[q, p, tc, j, e] = x[2q+j, tc*128+p, e]  (pair-interleaved)
    xnat = np.ascontiguousarray(
        xb.reshape(BS // 2, 2, tc_n, P, U).transpose(0, 3, 2, 1, 4)
    )
    # xtr[b, p, ec, t] = x[b, t, ec*128+p]
    xtr = np.ascontiguousarray(
        xb.reshape(BS, TE, EC, P).transpose(0, 3, 2, 1)
    )
    return xnat, xtr


def build_in_maps(all_inputs):
    """Full host prep: dict of the reference's 16 inputs -> per-core in_maps."""
    args = {k: np.asarray(v, dtype=np.float32) for k, v in all_inputs.items()
            if k != "x_seq"}
    x_seq = np.asarray(all_inputs["x_seq"], dtype=np.float32)
    per_core = _host_prep(**args)
    in_maps = []
    for c in range(N_CORES):
        m = dict(per_core[c])
        m["xnat"], m["xtr"] = _prep_x(x_seq[c * BS : (c + 1) * BS])
        in_maps.append(m)
    return in_maps


def kernel(inputs, h_tm, x_seq, V_a, W_a, U_a, b_a, C_z, W_z, b_z,
           C_r, W_r, b_r, C_p, U_p, b_p):
    from concourse.bass_utils import run_bass_kernel_spmd

    in_maps = build_in_maps(dict(
        inputs=inputs, h_tm=h_tm, x_seq=x_seq, V_a=V_a, W_a=W_a, U_a=U_a,
        b_a=b_a, C_z=C_z, W_z=W_z, b_z=b_z, C_r=C_r, W_r=W_r, b_r=b_r,
        C_p=C_p, U_p=U_p, b_p=b_p))
    nc = build_nc()
    res = run_bass_kernel_spmd(nc, in_maps, core_ids=list(range(N_CORES)))
    return np.concatenate([res.results[c]["ht"] for c in range(N_CORES)], axis=0)


# revision 48
# speedup vs baseline: 1.1514x; 1.0019x over previous
"""AttentionRNNCell Trainium2 kernel (v2).

Math (per batch row b):
  et[t]  = V_a . tanh( (h W_a + b_a) + x[t] U_a )        t in [0, TE)
  at     = exp(et);  s = sum(at)
  ctx    = (sum_t at[t] x[t]) / s
  zt     = sigmoid(h W_z + [inp, ctx] C_z + b_z)
  rt     = sigmoid(h W_r + [inp, ctx] C_r + b_r)
  tht    = tanh((rt*h) U_p + [inp, ctx] C_p + b_p)
  ht     = (1-zt)*h + zt*tht

Distribution: data-parallel over batch B=128 across 8 cores (16 rows each).

v2 vs v1: the on-chip SBUF->SBUF DMA transposes of x (512 per core, ~635us
serialized through the Sync engine) are gone -- the host ships x twice in
bf16, pre-tiled in both layouts the PE needs:
  - xnat[b, p, tc, e] = x[b, tc*128+p, e]   (t on partitions, ctx rhs)
  - xtr [b, p, ec, t] = x[b, t, ec*128+p]   (e on partitions, uxpb rhs)
Context is computed with x as the MOVING operand (lhsT = at column, M=1,
N=256) -- 16 matmuls/row instead of 32 stationary x-tile loads + N=1
matmuls. Everything not depending on x_seq is folded on host as in v1.
"""

from contextlib import ExitStack

import numpy as np
import ml_dtypes

import concourse.bass as bass
import concourse.mybir as mybir
import concourse.tile as tile

BF16 = ml_dtypes.bfloat16
NPF8 = ml_dtypes.float8_e4m3
F32 = mybir.dt.float32
BF = mybir.dt.bfloat16
F8 = mybir.dt.float8e4
DR = mybir.MatmulPerfMode.DoubleRow
AF = mybir.ActivationFunctionType

B, TE, U, IN_DIM = 128, 2048, 256, 256
N_CORES = 8
BS = B // N_CORES  # 16 batch rows per core
P = 128
EC = U // P  # e-chunks (2)
UC = U // P  # u-chunks (2)


def split_multi_waits(nc, max_waits=1):
    """This container's walrus rejects instructions carrying more than one
    sync wait. Hoist extra waits onto standalone same-engine NoOps inserted
    immediately before the offending instruction (semantically identical:
    the engine blocks on each wait in order before executing it)."""
    n_new = 0
    for f in nc.m.functions:
        for blk in f.blocks:
            new_insts = []
            for inst in blk.instructions:
                si = inst.sync_info
                waits = list(si.on_wait) if si and si.on_wait else []
                if len(waits) > max_waits:
                    for w in waits[:-max_waits]:
                        nop = mybir.InstNoOp(
                            name=f"{inst.name}-hw{n_new}", ins=[], outs=[]
                        )
                        nop.engine = inst.engine
                        nop.sync_info = mybir.SyncInfo(on_wait=[w], on_update=[])
                        new_insts.append(nop)
                        n_new += 1
                    si.on_wait = waits[-max_waits:]
                new_insts.append(inst)
            blk.instructions = new_insts
    return n_new


def build_nc(bs=BS, te=TE, split_waits=True, debug_outs=False):
    """Build the per-core Bass module. Parametrized so a small variant can be
    simulated quickly; the production shape is (bs=16, te=2048)."""
    tc_n = te // P      # 128-col t-chunks (16)
    th_n = 2            # t halves (uxpb PSUM tile = [128, te/2] fp32, 2 banks)
    t_half = te // th_n
    tq_n = t_half // P  # 128-col chunks per half (8)

    nc = bass.Bass()
    xnat_d = nc.declare_dram_parameter("xnat", [bs // 2, P, tc_n, 2, U], F8, isOutput=False)
    xtr_d = nc.declare_dram_parameter("xtr", [bs, P, EC, te], F8, isOutput=False)
    ua_d = nc.declare_dram_parameter("ua", [U, U], F8, isOutput=False)
    va_d = nc.declare_dram_parameter("va", [U, 1], F8, isOutput=False)
    wxpbT_d = nc.declare_dram_parameter("wxpbT", [U, bs], F32, isOutput=False)
    hT_d = nc.declare_dram_parameter("hT", [U, bs], F32, isOutput=False)
    g0T_d = nc.declare_dram_parameter("g0T", [3, U, bs], F32, isOutput=False)
    cz_d = nc.declare_dram_parameter("cz", [U, U], F32, isOutput=False)
    cr_d = nc.declare_dram_parameter("cr", [U, U], F32, isOutput=False)
    cp_d = nc.declare_dram_parameter("cp", [U, U], F32, isOutput=False)
    up_d = nc.declare_dram_parameter("up", [U, U], F32, isOutput=False)
    id_d = nc.declare_dram_parameter("ident", [P, P], F32, isOutput=False)
    ht_d = nc.declare_dram_parameter("ht", [bs, U], F32, isOutput=True)
    if debug_outs:
        dbg_ctx_d = nc.declare_dram_parameter("dbg_ctx", [bs, U], F32, isOutput=True)
        dbg_es_d = nc.declare_dram_parameter("dbg_expsum", [P, bs], F32, isOutput=True)
        dbg_at_d = nc.declare_dram_parameter("dbg_at", [P, te // P], F32, isOutput=True)

    with tile.TileContext(nc) as tc, ExitStack() as ctx:
        singles = ctx.enter_context(tc.tile_pool(name="singles", bufs=1))
        xnat_p = ctx.enter_context(tc.tile_pool(name="xnat", bufs=4))
        xtr_p = ctx.enter_context(tc.tile_pool(name="xtr", bufs=3))
        tanh_p = ctx.enter_context(tc.tile_pool(name="tanh", bufs=8))
        at_p = ctx.enter_context(tc.tile_pool(name="at", bufs=4))
        small_p = ctx.enter_context(tc.tile_pool(name="small", bufs=4))
        uxpb_ps = ctx.enter_context(tc.tile_pool(name="uxpbps", bufs=3, space="PSUM"))
        et_ps = ctx.enter_context(tc.tile_pool(name="etps", bufs=1, space="PSUM"))
        ctx_ps = ctx.enter_context(tc.tile_pool(name="ctxps", bufs=1, space="PSUM"))

        # ---- setup: weights / small per-core tensors ----
        # Only ua/va/wxpb gate the first row's compute; everything else is
        # tail-only and loads on the (startup-idle) ACT queue after the
        # first rows' x DMAs are in flight.
        ua_sb = singles.tile([P, EC, U], F8)
        nc.sync.dma_start(out=ua_sb, in_=ua_d[:, :].rearrange("(c p) u -> p c u", p=P))
        va_sb = singles.tile([P, UC, 1], F8)
        nc.sync.dma_start(out=va_sb, in_=va_d[:, :].rearrange("(c p) o -> p c o", p=P))
        wxpb_sb = singles.tile([P, UC, bs], F32)
        nc.sync.dma_start(out=wxpb_sb, in_=wxpbT_d[:, :].rearrange("(c p) b -> p c b", p=P))
        hT_sb = singles.tile([P, UC, bs], F32)
        g0_sb = singles.tile([P, 3, UC, bs], F32)
        gate_w = {}
        for name in ("cz", "cr", "cp", "up"):
            gate_w[name] = singles.tile([P, EC, U], F32, name=f"{name}_sb")
        id_sb = singles.tile([P, P], F32)
        ones_sb = singles.tile([P, P], F32)
        nc.vector.memset(ones_sb, 1.0)

        def load_tail_weights():
            nc.sync.dma_start(out=hT_sb, in_=hT_d[:, :].rearrange("(c p) b -> p c b", p=P))
            nc.sync.dma_start(out=g0_sb, in_=g0T_d[:, :, :].rearrange("g (c p) b -> p g c b", p=P))
            for name, d in (("cz", cz_d), ("cr", cr_d), ("cp", cp_d), ("up", up_d)):
                nc.sync.dma_start(out=gate_w[name], in_=d[:, :].rearrange("(c p) u -> p c u", p=P))
            nc.sync.dma_start(out=id_sb, in_=id_d[:, :])
        expsum_all = singles.tile([P, bs], F32)
        ctx_rows = singles.tile([bs, U], F32)  # unnormalized ctx, one row per b
        # Block-diagonal at tiles for the paired-ctx DoubleRow: slot [j, m]
        # holds row (2q+j)'s at iff j == m, else stays the zero written once
        # here. Two tiles ping-pong across pairs.
        # [p, j, tc, m] layout: the k-tile (j) stride is tc_n*2 bytes -- the
        # dual-fp8 ldweights ISA requires k-tile stride >= 16 bytes.
        at2_tiles = []
        for i in range(2):
            at2 = singles.tile([P, 2, tc_n, 2], F8, name=f"at2_{i}")
            nc.vector.memset(at2, 0.0)
            at2_tiles.append(at2)

        # ---- streaming loop over batch rows, software-pipelined one deep.
        # Per iteration the issue order is: [et(b-1), exp(b-1)] ->
        # [uxpb(b), tanh(b)] -> [ctx(b-1)], so every PE instruction's
        # producer (ACT tanh/exp of the PREVIOUS row) has a full row of PE
        # work to hide behind -- the in-order PE queue never stalls.
        def stage_dma(b):
            if b == 0:
                # Row 0 only: four one-shot chunk tiles on three queues, so
                # each first-row uxpb matmul depends on exactly one small DMA
                # (deps are tile-granular) -- first matmul fires ~15us sooner.
                xt = [
                    singles.tile([P, EC, 512], F8, name=f"xt0_{c}")
                    for c in range(4)
                ]
                engs = [nc.sync, nc.scalar, nc.gpsimd, nc.sync]
                for c in range(4):
                    engs[c].dma_start(
                        out=xt[c], in_=xtr_d[b, :, :, c * 512 : (c + 1) * 512]
                    )
            else:
                xt = xtr_p.tile([P, EC, te], F8, tag="xt", name=f"xt{b}")
                nc.sync.dma_start(out=xt[:, :, 0:t_half], in_=xtr_d[b, :, :, 0:t_half])
                nc.gpsimd.dma_start(out=xt[:, :, t_half:te], in_=xtr_d[b, :, :, t_half:te])
            x_nat = None
            if b % 2 == 0:  # x for ctx lands pair-interleaved, one tile per pair
                x_nat = xnat_p.tile([P, tc_n, 2, U], F8, tag="xnat", name=f"xnat{b}")
                nc.gpsimd.dma_start(out=x_nat, in_=xnat_d[b // 2])
            return x_nat, xt

        def stage_uxpb_tanh(b, xt):
            # uxpb: out[u, t] = sum_e ua[e, u] * xt[e, t] -- fp8 DoubleRow
            # contracts both e-chunks in one matmul. tanh (per-partition
            # bias) -> SBUF fp8 tiles shaped [u, uc, t] for the et DoubleRow.
            tanh_ts = []
            for th in range(th_n):
                tanh_t = tanh_p.tile([P, UC, t_half], F8, tag="tanh", name=f"th{th}")
                n_mm = min(512, t_half)
                uxs = [
                    uxpb_ps.tile([P, t_half], F32, tag="ux", name=f"ux{uc}{th}")
                    for uc in range(UC)
                ]
                for n0 in range(0, t_half, n_mm):
                    if isinstance(xt, list):
                        rhs = xt[(th * t_half + n0) // 512]
                    else:
                        rhs = xt[:, :, th * t_half + n0 : th * t_half + n0 + n_mm]
                    for uc in range(UC):
                        nc.tensor.matmul(
                            out=uxs[uc][:, n0 : n0 + n_mm],
                            lhsT=ua_sb[:, :, uc * P : (uc + 1) * P],
                            rhs=rhs,
                            perf_mode=DR,
                        )
                for uc in range(UC):
                    nc.scalar.activation(
                        out=tanh_t[:, uc, :], in_=uxs[uc], func=AF.Tanh,
                        bias=wxpb_sb[:, uc, b : b + 1],
                    )
                tanh_ts.append(tanh_t)
            return tanh_ts

        def stage_et_exp(b, tanh_ts):
            et = et_ps.tile([P, tc_n], F32, tag="etps", name=f"et{b}")
            for th in range(th_n):
                for tq in range(tq_n):
                    nc.tensor.matmul(
                        out=et[:, th * tq_n + tq : th * tq_n + tq + 1],
                        lhsT=tanh_ts[th][:, :, tq * P : (tq + 1) * P],
                        rhs=va_sb,
                        perf_mode=DR,
                    )
            # exp lands on the diagonal slot of the pair's block-diag tile
            j = b % 2
            at2 = at2_tiles[(b // 2) % 2]
            nc.scalar.activation(
                out=at2[:, j, :, j], in_=et, func=AF.Exp,
                accum_out=expsum_all[:, b : b + 1],
            )
            if debug_outs and b == 0:
                at_f32 = small_p.tile([P, tc_n], F32, name="at_f32")
                nc.vector.tensor_copy(at_f32, at2[:, 0, :, 0])
                nc.sync.dma_start(out=dbg_at_d[:, :], in_=at_f32)
            return at2

        def stage_ctx_pair(q, at2, x_nat):
            # Paired ctx: block-diagonal at2 on the two k-tiles against the
            # pair-interleaved x tile -> out[m, e] = row (2q+m)'s ctx partial.
            cps = ctx_ps.tile([2, U], F32, tag="ctxps", name=f"cps{q}")
            for tcc in range(tc_n):
                nc.tensor.matmul(
                    out=cps,
                    lhsT=at2[:, :, tcc, :],
                    rhs=x_nat[:, tcc, :, :],
                    start=(tcc == 0),
                    stop=(tcc == tc_n - 1),
                    perf_mode=DR,
                )
            stg = small_p.tile([2, U], F32, tag="ctxstg", name=f"stg{q}")
            nc.vector.tensor_copy(stg, cps)
            nc.sync.dma_start(out=ctx_rows[2 * q : 2 * q + 2, :], in_=stg)

        prev = None  # (b, tanh_ts)
        pair_xnat = {}
        for b in range(bs):
            x_nat, xt = stage_dma(b)
            if x_nat is not None:
                pair_xnat[b // 2] = x_nat
            if b == 1:
                load_tail_weights()
            if prev is not None:
                at2_prev = stage_et_exp(prev[0], prev[1])
                pb = prev[0]
            tanh_ts = stage_uxpb_tanh(b, xt)
            if prev is not None and pb % 2 == 1:
                stage_ctx_pair(pb // 2, at2_prev, pair_xnat.pop(pb // 2))
            prev = (b, tanh_ts)
        at2_last = stage_et_exp(prev[0], prev[1])
        stage_ctx_pair(prev[0] // 2, at2_last, pair_xnat.pop(prev[0] // 2))

        # ---- tail: normalize context, gates, output ----
        if debug_outs:
            nc.sync.dma_start(out=dbg_ctx_d[:, :], in_=ctx_rows)
            nc.sync.dma_start(out=dbg_es_d[:, :], in_=expsum_all)
        s_ps = et_ps.tile([P, bs], F32, tag="etps", name="s_ps")
        nc.tensor.matmul(out=s_ps, lhsT=ones_sb, rhs=expsum_all)
        recips = small_p.tile([P, bs], F32)
        nc.vector.reciprocal(recips, s_ps)
        # ctx_rows [bs, U] -> ctxT [e%128, ec, b] via PE transpose; normalize
        # by 1/s on the way out of PSUM.
        ctxn = singles.tile([P, EC, bs], F32)
        for e in range(EC):
            tp = et_ps.tile([P, bs], F32, tag="etps", name=f"ctxT{e}")
            nc.tensor.transpose(tp, ctx_rows[:, e * P : (e + 1) * P], id_sb[0:bs, 0:bs])
            nc.vector.tensor_mul(ctxn[:, e, :], tp, recips)

        def gate_psum(w_names_rhs, name):
            """psum[uc] = sum over (w, rhs) pairs of w^T @ rhs, per u-chunk."""
            outs = []
            for uc in range(UC):
                g = et_ps.tile([P, bs], F32, tag="etps", name=f"{name}{uc}")
                n_mm = sum(EC for _ in w_names_rhs)
                i = 0
                for w_sb, rhs_fn in w_names_rhs:
                    for e in range(EC):
                        nc.tensor.matmul(
                            out=g,
                            lhsT=w_sb[:, e, uc * P : (uc + 1) * P],
                            rhs=rhs_fn(e),
                            start=(i == 0),
                            stop=(i == n_mm - 1),
                        )
                        i += 1
                outs.append(g)
            return outs

        # zt^T, rt^T = sigmoid(g0 + C_*ctx^T ctx^T)
        zt_sb = small_p.tile([P, UC, bs], F32)
        rt_sb = small_p.tile([P, UC, bs], F32)
        for gi, (wname, dst) in enumerate((("cz", zt_sb), ("cr", rt_sb))):
            gps = gate_psum([(gate_w[wname], lambda e: ctxn[:, e, :])], wname)
            for uc in range(UC):
                tmp = small_p.tile([P, bs], F32, tag="gtmp", name=f"t{wname}{uc}")
                nc.vector.tensor_add(tmp, gps[uc], g0_sb[:, gi, uc, :])
                nc.scalar.activation(out=dst[:, uc, :], in_=tmp, func=AF.Sigmoid)

        # rh^T = rt^T * h^T ; tht^T = tanh(g0p + U_p^T rh^T + C_pctx^T ctx^T)
        rh_sb = small_p.tile([P, UC, bs], F32)
        for uc in range(UC):
            nc.vector.tensor_mul(rh_sb[:, uc, :], rt_sb[:, uc, :], hT_sb[:, uc, :])
        gps = gate_psum(
            [(gate_w["up"], lambda e: rh_sb[:, e, :]), (gate_w["cp"], lambda e: ctxn[:, e, :])],
            "cp",
        )
        ht_nat = small_p.tile([bs, U], F32)
        for uc in range(UC):
            tmp = small_p.tile([P, bs], F32, tag="gtmp", name=f"tp{uc}")
            nc.vector.tensor_add(tmp, gps[uc], g0_sb[:, 2, uc, :])
            tht = small_p.tile([P, bs], F32, tag="gtmp", name=f"tht{uc}")
            nc.scalar.activation(out=tht, in_=tmp, func=AF.Tanh)
            # ht^T = h^T + zt^T*(tht^T - h^T)
            nc.vector.tensor_sub(tht, tht, hT_sb[:, uc, :])
            nc.vector.tensor_mul(tht, tht, zt_sb[:, uc, :])
            nc.vector.tensor_add(tht, tht, hT_sb[:, uc, :])
            tp = et_ps.tile([bs, P], F32, tag="etps", name=f"htp{uc}")
            nc.tensor.transpose(tp, tht, id_sb)
            nc.vector.tensor_copy(ht_nat[:, uc * P : (uc + 1) * P], tp)
        nc.sync.dma_start(out=ht_d[:, :], in_=ht_nat)

    if split_waits:
        split_multi_waits(nc)
    return nc


def _host_prep(inputs, h_tm, V_a, W_a, U_a, b_a, C_z, W_z, b_z, C_r, W_r, b_r,
               C_p, U_p, b_p):
    """Fold everything not depending on x_seq into small per-core tensors."""
    wxpb = h_tm @ W_a + b_a                                # [B, U]
    g_z0 = h_tm @ W_z + inputs @ C_z[:IN_DIM] + b_z        # [B, U]
    g_r0 = h_tm @ W_r + inputs @ C_r[:IN_DIM] + b_r
    g_p0 = inputs @ C_p[:IN_DIM] + b_p
    shared = {
        "ua": np.ascontiguousarray(U_a.astype(NPF8)),
        "va": np.ascontiguousarray(V_a.reshape(U, 1).astype(NPF8)),
        "cz": np.ascontiguousarray(C_z[IN_DIM:].astype(np.float32)),
        "cr": np.ascontiguousarray(C_r[IN_DIM:].astype(np.float32)),
        "cp": np.ascontiguousarray(C_p[IN_DIM:].astype(np.float32)),
        "up": np.ascontiguousarray(U_p.astype(np.float32)),
        "ident": np.eye(P, dtype=np.float32),
    }
    per_core = []
    for c in range(N_CORES):
        s = slice(c * BS, (c + 1) * BS)
        per_core.append(
            {
                "wxpbT": np.ascontiguousarray(wxpb[s].T.astype(np.float32)),
                "hT": np.ascontiguousarray(h_tm[s].T.astype(np.float32)),
                "g0T": np.ascontiguousarray(
                    np.stack([g_z0[s].T, g_r0[s].T, g_p0[s].T]).astype(np.float32)
                ),
                **shared,
            }
        )
    return per_core


def _prep_x(x_core):
    """Pre-tile one core's x [bs, TE, U] into both fp8 layouts."""
    xb = x_core.astype(NPF8)
    tc_n = TE // P
    # xnat[q, p, tc, j, e] = x[2q+j, tc*128+p, e]  (pair-interleaved)
    xnat = np.ascontiguousarray(
        xb.reshape(BS // 2, 2, tc_n, P, U).transpose(0, 3, 2, 1, 4)
    )
    # xtr[b, p, ec, t] = x[b, t, ec*128+p]
    xtr = np.ascontiguousarray(
        xb.reshape(BS, TE, EC, P).transpose(0, 3, 2, 1)
    )
    return xnat, xtr


def build_in_maps(all_inputs):
    """Full host prep: dict of the reference's 16 inputs -> per-core in_maps."""
    args = {k: np.asarray(v, dtype=np.float32) for k, v in all_inputs.items()
            if k != "x_seq"}
    x_seq = np.asarray(all_inputs["x_seq"], dtype=np.float32)
    per_core = _host_prep(**args)
    in_maps = []
    for c in range(N_CORES):
        m = dict(per_core[c])
        m["xnat"], m["xtr"] = _prep_x(x_seq[c * BS : (c + 1) * BS])
        in_maps.append(m)
    return in_maps


def kernel(inputs, h_tm, x_seq, V_a, W_a, U_a, b_a, C_z, W_z, b_z,
           C_r, W_r, b_r, C_p, U_p, b_p):
    from concourse.bass_utils import run_bass_kernel_spmd

    in_maps = build_in_maps(dict(
        inputs=inputs, h_tm=h_tm, x_seq=x_seq, V_a=V_a, W_a=W_a, U_a=U_a,
        b_a=b_a, C_z=C_z, W_z=W_z, b_z=b_z, C_r=C_r, W_r=W_r, b_r=b_r,
        C_p=C_p, U_p=U_p, b_p=b_p))
    nc = build_nc()
    res = run_bass_kernel_spmd(nc, in_maps, core_ids=list(range(N_CORES)))
    return np.concatenate([res.results[c]["ht"] for c in range(N_CORES)], axis=0)


# revision 49
# speedup vs baseline: 1.1599x; 1.0074x over previous
"""AttentionRNNCell Trainium2 kernel (v2).

Math (per batch row b):
  et[t]  = V_a . tanh( (h W_a + b_a) + x[t] U_a )        t in [0, TE)
  at     = exp(et);  s = sum(at)
  ctx    = (sum_t at[t] x[t]) / s
  zt     = sigmoid(h W_z + [inp, ctx] C_z + b_z)
  rt     = sigmoid(h W_r + [inp, ctx] C_r + b_r)
  tht    = tanh((rt*h) U_p + [inp, ctx] C_p + b_p)
  ht     = (1-zt)*h + zt*tht

Distribution: data-parallel over batch B=128 across 8 cores (16 rows each).

v2 vs v1: the on-chip SBUF->SBUF DMA transposes of x (512 per core, ~635us
serialized through the Sync engine) are gone -- the host ships x twice in
bf16, pre-tiled in both layouts the PE needs:
  - xnat[b, p, tc, e] = x[b, tc*128+p, e]   (t on partitions, ctx rhs)
  - xtr [b, p, ec, t] = x[b, t, ec*128+p]   (e on partitions, uxpb rhs)
Context is computed with x as the MOVING operand (lhsT = at column, M=1,
N=256) -- 16 matmuls/row instead of 32 stationary x-tile loads + N=1
matmuls. Everything not depending on x_seq is folded on host as in v1.
"""

from contextlib import ExitStack

import numpy as np
import ml_dtypes

import concourse.bass as bass
import concourse.mybir as mybir
import concourse.tile as tile

BF16 = ml_dtypes.bfloat16
NPF8 = ml_dtypes.float8_e4m3
F32 = mybir.dt.float32
BF = mybir.dt.bfloat16
F8 = mybir.dt.float8e4
DR = mybir.MatmulPerfMode.DoubleRow
AF = mybir.ActivationFunctionType

B, TE, U, IN_DIM = 128, 2048, 256, 256
N_CORES = 8
BS = B // N_CORES  # 16 batch rows per core
P = 128
EC = U // P  # e-chunks (2)
UC = U // P  # u-chunks (2)


def split_multi_waits(nc, max_waits=1):
    """This container's walrus rejects instructions carrying more than one
    sync wait. Hoist extra waits onto standalone same-engine NoOps inserted
    immediately before the offending instruction (semantically identical:
    the engine blocks on each wait in order before executing it)."""
    n_new = 0
    for f in nc.m.functions:
        for blk in f.blocks:
            new_insts = []
            for inst in blk.instructions:
                si = inst.sync_info
                waits = list(si.on_wait) if si and si.on_wait else []
                if len(waits) > max_waits:
                    for w in waits[:-max_waits]:
                        nop = mybir.InstNoOp(
                            name=f"{inst.name}-hw{n_new}", ins=[], outs=[]
                        )
                        nop.engine = inst.engine
                        nop.sync_info = mybir.SyncInfo(on_wait=[w], on_update=[])
                        new_insts.append(nop)
                        n_new += 1
                    si.on_wait = waits[-max_waits:]
                new_insts.append(inst)
            blk.instructions = new_insts
    return n_new


def build_nc(bs=BS, te=TE, split_waits=True, debug_outs=False):
    """Build the per-core Bass module. Parametrized so a small variant can be
    simulated quickly; the production shape is (bs=16, te=2048)."""
    tc_n = te // P      # 128-col t-chunks (16)
    th_n = 2            # t halves (uxpb PSUM tile = [128, te/2] fp32, 2 banks)
    t_half = te // th_n
    tq_n = t_half // P  # 128-col chunks per half (8)

    nc = bass.Bass()
    xnat_d = nc.declare_dram_parameter("xnat", [bs // 2, P, tc_n, 2, U], F8, isOutput=False)
    xtr_d = nc.declare_dram_parameter("xtr", [bs, P, EC, te], F8, isOutput=False)
    ua_d = nc.declare_dram_parameter("ua", [U, U], F8, isOutput=False)
    va_d = nc.declare_dram_parameter("va", [U, 1], F8, isOutput=False)
    wxpbT_d = nc.declare_dram_parameter("wxpbT", [U, bs], F32, isOutput=False)
    hT_d = nc.declare_dram_parameter("hT", [U, bs], F32, isOutput=False)
    g0T_d = nc.declare_dram_parameter("g0T", [3, U, bs], F32, isOutput=False)
    cz_d = nc.declare_dram_parameter("cz", [U, U], F32, isOutput=False)
    cr_d = nc.declare_dram_parameter("cr", [U, U], F32, isOutput=False)
    cp_d = nc.declare_dram_parameter("cp", [U, U], F32, isOutput=False)
    up_d = nc.declare_dram_parameter("up", [U, U], F32, isOutput=False)
    id_d = nc.declare_dram_parameter("ident", [P, P], F32, isOutput=False)
    ht_d = nc.declare_dram_parameter("ht", [bs, U], F32, isOutput=True)
    if debug_outs:
        dbg_ctx_d = nc.declare_dram_parameter("dbg_ctx", [bs, U], F32, isOutput=True)
        dbg_es_d = nc.declare_dram_parameter("dbg_expsum", [P, bs], F32, isOutput=True)
        dbg_at_d = nc.declare_dram_parameter("dbg_at", [P, te // P], F32, isOutput=True)

    with tile.TileContext(nc) as tc, ExitStack() as ctx:
        singles = ctx.enter_context(tc.tile_pool(name="singles", bufs=1))
        xnat_p = ctx.enter_context(tc.tile_pool(name="xnat", bufs=4))
        xtr_p = ctx.enter_context(tc.tile_pool(name="xtr", bufs=3))
        tanh_p = ctx.enter_context(tc.tile_pool(name="tanh", bufs=8))
        at_p = ctx.enter_context(tc.tile_pool(name="at", bufs=4))
        small_p = ctx.enter_context(tc.tile_pool(name="small", bufs=4))
        uxpb_ps = ctx.enter_context(tc.tile_pool(name="uxpbps", bufs=3, space="PSUM"))
        et_ps = ctx.enter_context(tc.tile_pool(name="etps", bufs=1, space="PSUM"))
        ctx_ps = ctx.enter_context(tc.tile_pool(name="ctxps", bufs=1, space="PSUM"))

        # ---- setup: weights / small per-core tensors ----
        # Only ua/va/wxpb gate the first row's compute; everything else is
        # tail-only and loads on the (startup-idle) ACT queue after the
        # first rows' x DMAs are in flight.
        ua_sb = singles.tile([P, EC, U], F8)
        va_sb = singles.tile([P, UC, 1], F8)
        wxpb_sb = singles.tile([P, UC, bs], F32)

        def load_first_weights():
            # Issued AFTER row 0's xt chunk DMAs: keeps the first uxpb
            # matmul's data chunk at the head of the sync queue.
            nc.sync.dma_start(out=ua_sb, in_=ua_d[:, :].rearrange("(c p) u -> p c u", p=P))
            nc.scalar.dma_start(out=va_sb, in_=va_d[:, :].rearrange("(c p) o -> p c o", p=P))
            nc.scalar.dma_start(out=wxpb_sb, in_=wxpbT_d[:, :].rearrange("(c p) b -> p c b", p=P))
        hT_sb = singles.tile([P, UC, bs], F32)
        g0_sb = singles.tile([P, 3, UC, bs], F32)
        gate_w = {}
        for name in ("cz", "cr", "cp", "up"):
            gate_w[name] = singles.tile([P, EC, U], F32, name=f"{name}_sb")
        id_sb = singles.tile([P, P], F32)
        ones_sb = singles.tile([P, P], F32)
        nc.vector.memset(ones_sb, 1.0)

        def load_tail_weights():
            nc.sync.dma_start(out=hT_sb, in_=hT_d[:, :].rearrange("(c p) b -> p c b", p=P))
            nc.sync.dma_start(out=g0_sb, in_=g0T_d[:, :, :].rearrange("g (c p) b -> p g c b", p=P))
            for name, d in (("cz", cz_d), ("cr", cr_d), ("cp", cp_d), ("up", up_d)):
                nc.sync.dma_start(out=gate_w[name], in_=d[:, :].rearrange("(c p) u -> p c u", p=P))
            nc.sync.dma_start(out=id_sb, in_=id_d[:, :])
        expsum_all = singles.tile([P, bs], F32)
        ctx_rows = singles.tile([bs, U], F32)  # unnormalized ctx, one row per b
        # Block-diagonal at tiles for the paired-ctx DoubleRow: slot [j, m]
        # holds row (2q+j)'s at iff j == m, else stays the zero written once
        # here. Two tiles ping-pong across pairs.
        # [p, j, tc, m] layout: the k-tile (j) stride is tc_n*2 bytes -- the
        # dual-fp8 ldweights ISA requires k-tile stride >= 16 bytes.
        at2_tiles = []
        for i in range(2):
            at2 = singles.tile([P, 2, tc_n, 2], F8, name=f"at2_{i}")
            nc.vector.memset(at2, 0.0)
            at2_tiles.append(at2)

        # ---- streaming loop over batch rows, software-pipelined one deep.
        # Per iteration the issue order is: [et(b-1), exp(b-1)] ->
        # [uxpb(b), tanh(b)] -> [ctx(b-1)], so every PE instruction's
        # producer (ACT tanh/exp of the PREVIOUS row) has a full row of PE
        # work to hide behind -- the in-order PE queue never stalls.
        def stage_dma(b):
            if b == 0:
                # Row 0 only: four one-shot chunk tiles on three queues, so
                # each first-row uxpb matmul depends on exactly one small DMA
                # (deps are tile-granular) -- first matmul fires ~15us sooner.
                xt = [
                    singles.tile([P, EC, 512], F8, name=f"xt0_{c}")
                    for c in range(4)
                ]
                engs = [nc.sync, nc.scalar, nc.gpsimd, nc.sync]
                for c in range(4):
                    engs[c].dma_start(
                        out=xt[c], in_=xtr_d[b, :, :, c * 512 : (c + 1) * 512]
                    )
            else:
                xt = xtr_p.tile([P, EC, te], F8, tag="xt", name=f"xt{b}")
                nc.sync.dma_start(out=xt[:, :, 0:t_half], in_=xtr_d[b, :, :, 0:t_half])
                nc.gpsimd.dma_start(out=xt[:, :, t_half:te], in_=xtr_d[b, :, :, t_half:te])
            x_nat = None
            if b % 2 == 0:  # x for ctx lands pair-interleaved, one tile per pair
                x_nat = xnat_p.tile([P, tc_n, 2, U], F8, tag="xnat", name=f"xnat{b}")
                nc.gpsimd.dma_start(out=x_nat, in_=xnat_d[b // 2])
            return x_nat, xt

        def stage_uxpb_tanh(b, xt):
            # uxpb: out[u, t] = sum_e ua[e, u] * xt[e, t] -- fp8 DoubleRow
            # contracts both e-chunks in one matmul. tanh (per-partition
            # bias) -> SBUF fp8 tiles shaped [u, uc, t] for the et DoubleRow.
            tanh_ts = []
            for th in range(th_n):
                tanh_t = tanh_p.tile([P, UC, t_half], F8, tag="tanh", name=f"th{th}")
                n_mm = min(512, t_half)
                uxs = [
                    uxpb_ps.tile([P, t_half], F32, tag="ux", name=f"ux{uc}{th}")
                    for uc in range(UC)
                ]
                for n0 in range(0, t_half, n_mm):
                    if isinstance(xt, list):
                        rhs = xt[(th * t_half + n0) // 512]
                    else:
                        rhs = xt[:, :, th * t_half + n0 : th * t_half + n0 + n_mm]
                    for uc in range(UC):
                        nc.tensor.matmul(
                            out=uxs[uc][:, n0 : n0 + n_mm],
                            lhsT=ua_sb[:, :, uc * P : (uc + 1) * P],
                            rhs=rhs,
                            perf_mode=DR,
                        )
                for uc in range(UC):
                    nc.scalar.activation(
                        out=tanh_t[:, uc, :], in_=uxs[uc], func=AF.Tanh,
                        bias=wxpb_sb[:, uc, b : b + 1],
                    )
                tanh_ts.append(tanh_t)
            return tanh_ts

        def stage_et_exp(b, tanh_ts):
            et = et_ps.tile([P, tc_n], F32, tag="etps", name=f"et{b}")
            for th in range(th_n):
                for tq in range(tq_n):
                    nc.tensor.matmul(
                        out=et[:, th * tq_n + tq : th * tq_n + tq + 1],
                        lhsT=tanh_ts[th][:, :, tq * P : (tq + 1) * P],
                        rhs=va_sb,
                        perf_mode=DR,
                    )
            # exp lands on the diagonal slot of the pair's block-diag tile
            j = b % 2
            at2 = at2_tiles[(b // 2) % 2]
            nc.scalar.activation(
                out=at2[:, j, :, j], in_=et, func=AF.Exp,
                accum_out=expsum_all[:, b : b + 1],
            )
            if debug_outs and b == 0:
                at_f32 = small_p.tile([P, tc_n], F32, name="at_f32")
                nc.vector.tensor_copy(at_f32, at2[:, 0, :, 0])
                nc.sync.dma_start(out=dbg_at_d[:, :], in_=at_f32)
            return at2

        def stage_ctx_pair(q, at2, x_nat):
            # Paired ctx: block-diagonal at2 on the two k-tiles against the
            # pair-interleaved x tile -> out[m, e] = row (2q+m)'s ctx partial.
            cps = ctx_ps.tile([2, U], F32, tag="ctxps", name=f"cps{q}")
            for tcc in range(tc_n):
                nc.tensor.matmul(
                    out=cps,
                    lhsT=at2[:, :, tcc, :],
                    rhs=x_nat[:, tcc, :, :],
                    start=(tcc == 0),
                    stop=(tcc == tc_n - 1),
                    perf_mode=DR,
                )
            stg = small_p.tile([2, U], F32, tag="ctxstg", name=f"stg{q}")
            nc.vector.tensor_copy(stg, cps)
            nc.sync.dma_start(out=ctx_rows[2 * q : 2 * q + 2, :], in_=stg)

        prev = None  # (b, tanh_ts)
        pair_xnat = {}
        for b in range(bs):
            x_nat, xt = stage_dma(b)
            if b == 0:
                load_first_weights()
            if x_nat is not None:
                pair_xnat[b // 2] = x_nat
            if b == 1:
                load_tail_weights()
            if prev is not None:
                at2_prev = stage_et_exp(prev[0], prev[1])
                pb = prev[0]
            tanh_ts = stage_uxpb_tanh(b, xt)
            if prev is not None and pb % 2 == 1:
                stage_ctx_pair(pb // 2, at2_prev, pair_xnat.pop(pb // 2))
            prev = (b, tanh_ts)
        at2_last = stage_et_exp(prev[0], prev[1])
        stage_ctx_pair(prev[0] // 2, at2_last, pair_xnat.pop(prev[0] // 2))

        # ---- tail: normalize context, gates, output ----
        if debug_outs:
            nc.sync.dma_start(out=dbg_ctx_d[:, :], in_=ctx_rows)
            nc.sync.dma_start(out=dbg_es_d[:, :], in_=expsum_all)
        s_ps = et_ps.tile([P, bs], F32, tag="etps", name="s_ps")
        nc.tensor.matmul(out=s_ps, lhsT=ones_sb, rhs=expsum_all)
        recips = small_p.tile([P, bs], F32)
        nc.vector.reciprocal(recips, s_ps)
        # ctx_rows [bs, U] -> ctxT [e%128, ec, b] via PE transpose; normalize
        # by 1/s on the way out of PSUM.
        ctxn = singles.tile([P, EC, bs], F32)
        for e in range(EC):
            tp = et_ps.tile([P, bs], F32, tag="etps", name=f"ctxT{e}")
            nc.tensor.transpose(tp, ctx_rows[:, e * P : (e + 1) * P], id_sb[0:bs, 0:bs])
            nc.vector.tensor_mul(ctxn[:, e, :], tp, recips)

        def gate_psum(w_names_rhs, name):
            """psum[uc] = sum over (w, rhs) pairs of w^T @ rhs, per u-chunk."""
            outs = []
            for uc in range(UC):
                g = et_ps.tile([P, bs], F32, tag="etps", name=f"{name}{uc}")
                n_mm = sum(EC for _ in w_names_rhs)
                i = 0
                for w_sb, rhs_fn in w_names_rhs:
                    for e in range(EC):
                        nc.tensor.matmul(
                            out=g,
                            lhsT=w_sb[:, e, uc * P : (uc + 1) * P],
                            rhs=rhs_fn(e),
                            start=(i == 0),
                            stop=(i == n_mm - 1),
                        )
                        i += 1
                outs.append(g)
            return outs

        # zt^T, rt^T = sigmoid(g0 + C_*ctx^T ctx^T)
        zt_sb = small_p.tile([P, UC, bs], F32)
        rt_sb = small_p.tile([P, UC, bs], F32)
        for gi, (wname, dst) in enumerate((("cz", zt_sb), ("cr", rt_sb))):
            gps = gate_psum([(gate_w[wname], lambda e: ctxn[:, e, :])], wname)
            for uc in range(UC):
                tmp = small_p.tile([P, bs], F32, tag="gtmp", name=f"t{wname}{uc}")
                nc.vector.tensor_add(tmp, gps[uc], g0_sb[:, gi, uc, :])
                nc.scalar.activation(out=dst[:, uc, :], in_=tmp, func=AF.Sigmoid)

        # rh^T = rt^T * h^T ; tht^T = tanh(g0p + U_p^T rh^T + C_pctx^T ctx^T)
        rh_sb = small_p.tile([P, UC, bs], F32)
        for uc in range(UC):
            nc.vector.tensor_mul(rh_sb[:, uc, :], rt_sb[:, uc, :], hT_sb[:, uc, :])
        gps = gate_psum(
            [(gate_w["up"], lambda e: rh_sb[:, e, :]), (gate_w["cp"], lambda e: ctxn[:, e, :])],
            "cp",
        )
        ht_nat = small_p.tile([bs, U], F32)
        for uc in range(UC):
            tmp = small_p.tile([P, bs], F32, tag="gtmp", name=f"tp{uc}")
            nc.vector.tensor_add(tmp, gps[uc], g0_sb[:, 2, uc, :])
            tht = small_p.tile([P, bs], F32, tag="gtmp", name=f"tht{uc}")
            nc.scalar.activation(out=tht, in_=tmp, func=AF.Tanh)
            # ht^T = h^T + zt^T*(tht^T - h^T)
            nc.vector.tensor_sub(tht, tht, hT_sb[:, uc, :])
            nc.vector.tensor_mul(tht, tht, zt_sb[:, uc, :])
            nc.vector.tensor_add(tht, tht, hT_sb[:, uc, :])
            tp = et_ps.tile([bs, P], F32, tag="etps", name=f"htp{uc}")
            nc.tensor.transpose(tp, tht, id_sb)
            nc.vector.tensor_copy(ht_nat[:, uc * P : (uc + 1) * P], tp)
        nc.sync.dma_start(out=ht_d[:, :], in_=ht_nat)

    if split_waits:
        split_multi_waits(nc)
    return nc


def _host_prep(inputs, h_tm, V_a, W_a, U_a, b_a, C_z, W_z, b_z, C_r, W_r, b_r,
               C_p, U_p, b_p):
    """Fold everything not depending on x_seq into small per-core tensors."""
    wxpb = h_tm @ W_a + b_a                                # [B, U]
    g_z0 = h_tm @ W_z + inputs @ C_z[:IN_DIM] + b_z        # [B, U]
    g_r0 = h_tm @ W_r + inputs @ C_r[:IN_DIM] + b_r
    g_p0 = inputs @ C_p[:IN_DIM] + b_p
    shared = {
        "ua": np.ascontiguousarray(U_a.astype(NPF8)),
        "va": np.ascontiguousarray(V_a.reshape(U, 1).astype(NPF8)),
        "cz": np.ascontiguousarray(C_z[IN_DIM:].astype(np.float32)),
        "cr": np.ascontiguousarray(C_r[IN_DIM:].astype(np.float32)),
        "cp": np.ascontiguousarray(C_p[IN_DIM:].astype(np.float32)),
        "up": np.ascontiguousarray(U_p.astype(np.float32)),
        "ident": np.eye(P, dtype=np.float32),
    }
    per_core = []
    for c in range(N_CORES):
        s = slice(c * BS, (c + 1) * BS)
        per_core.append(
            {
                "wxpbT": np.ascontiguousarray(wxpb[s].T.astype(np.float32)),
                "hT": np.ascontiguousarray(h_tm[s].T.astype(np.float32)),
                "g0T": np.ascontiguousarray(
                    np.stack([g_z0[s].T, g_r0[s].T, g_p0[s].T]).astype(np.float32)
                ),
                **shared,
            }
        )
    return per_core


def _prep_x(x_core):
    """Pre-tile one core's x [bs, TE, U] into both fp8 layouts."""
    xb = x_core.astype(NPF8)
    tc_n = TE // P
    # xnat[q, p, tc, j, e] = x[2q+j, tc*128+p, e]  (pair-interleaved)
    xnat = np.ascontiguousarray(
        xb.reshape(BS // 2, 2, tc_n, P, U).transpose(0, 3, 2, 1, 4)
    )
    # xtr[b, p, ec, t] = x[b, t, ec*128+p]
    xtr = np.ascontiguousarray(
        xb.reshape(BS, TE, EC, P).transpose(0, 3, 2, 1)
    )
    return xnat, xtr


def build_in_maps(all_inputs):
    """Full host prep: dict of the reference's 16 inputs -> per-core in_maps."""
    args = {k: np.asarray(v, dtype=np.float32) for k, v in all_inputs.items()
            if k != "x_seq"}
    x_seq = np.asarray(all_inputs["x_seq"], dtype=np.float32)
    per_core = _host_prep(**args)
    in_maps = []
    for c in range(N_CORES):
        m = dict(per_core[c])
        m["xnat"], m["xtr"] = _prep_x(x_seq[c * BS : (c + 1) * BS])
        in_maps.append(m)
    return in_maps


def kernel(inputs, h_tm, x_seq, V_a, W_a, U_a, b_a, C_z, W_z, b_z,
           C_r, W_r, b_r, C_p, U_p, b_p):
    from concourse.bass_utils import run_bass_kernel_spmd

    in_maps = build_in_maps(dict(
        inputs=inputs, h_tm=h_tm, x_seq=x_seq, V_a=V_a, W_a=W_a, U_a=U_a,
        b_a=b_a, C_z=C_z, W_z=W_z, b_z=b_z, C_r=C_r, W_r=W_r, b_r=b_r,
        C_p=C_p, U_p=U_p, b_p=b_p))
    nc = build_nc()
    res = run_bass_kernel_spmd(nc, in_maps, core_ids=list(range(N_CORES)))
    return np.concatenate([res.results[c]["ht"] for c in range(N_CORES)], axis=0)
